# revision 1
# baseline (speedup 1.0000x reference)
"""4D Conv-MLP (conv3^4 -> ReLU -> conv3^4) on 8 Trainium2 NeuronCores.

Sharding: core = b*4 + j  (batch b in {0,1}, H-slab j in {0..3}, 8 output rows
each). Each core computes its output slab independently: conv1 is recomputed on
a 1-row h halo (10 h rows from 12 x rows), so no cross-core communication is
needed. One SPMD program for all cores; per-core boundary behavior is driven by
data (host-zeroed x halos + h halo-row masks).

On-chip algorithm (implicit GEMM over the 81 taps, fp16 operands, fp32 PSUM):
  - x lives channel-on-partition as zero-padded flat planes per t ([16 D][12 H]
    [34 W], +1 lead pad), in two SBUF tiles of two shifted copies each:
    tileA = (x, x+1) and tileB = (x+2, x+36), so most K=128 matmuls contract
    two taps at once.
  - conv1: per (t, d): N=340 matmuls; each valid (kt, ku) block = 4 K=128
    pairs + 1 K=64 single (optimal for a 3x3 (kv, kw) grid with shift deltas
    {1, 34}); all-zero T/D edge taps are skipped; ReLU+bias on the Scalar
    engine writes fp16 h (pads skipped, out-of-image halo rows masked).
  - conv2: N=512 runs over d-pairs (N=256 at D edges, pad taps skipped);
    taps alternate PE column groups via tile_position (0,0)/(0,64) so two
    M=64 matmuls run concurrently; halves summed + bias on Scalar/DVE.
  Known pitfall baked into the structure: two partial-row (K=64) matmuls
  must never be adjacent in the PE stream (device wedge), and tile_size
  transitions stall the LDWEIGHTS pipeline, so singles are batched at the
  end of each conv1 accumulation chain.
"""

import numpy as np

B, C_IN, C_HID, C_OUT = 2, 64, 128, 64
T, D, H, W = 4, 16, 32, 32
NCORES, NJ = 8, 4
SH = H // NJ          # 8 out rows per slab
XH = SH + 4           # 12 x rows per slab
HHH = SH + 2          # 10 h rows per slab
XROW = 34             # padded W
XDP = 12 * XROW       # 408
XP = 1 + 16 * XDP + 7   # x plane size (real D rows only) = 6536
HD, HW_ = 18, 34
HP = HD * HHH * HW_   # h plane = 6120
N1 = HHH * XROW       # conv1 run = 340
N2 = 512              # conv2 run (2 d-rows)

_cache = {}


def _t_taps(t):
    return [kt for kt in range(3) if 0 <= t + kt - 1 < T]


def _g27(kt, ku, kv):
    return (kt * 3 + ku) * 3 + kv


def _g81(kt, ku, kv, kw):
    return ((kt * 3 + ku) * 3 + kv) * 3 + kw


def _make_host_arrays(x, w1, b1, w2, b2):
    x = np.asarray(x, np.float32)
    Xs, MTs, MBs = [], [], []
    for core in range(NCORES):
        b, j = divmod(core, NJ)
        h0 = SH * j
        slab = np.zeros((C_IN, T, D, XH, W), np.float32)
        lo, hi = h0 - 2, h0 + 10
        slo, shi = max(lo, 0), min(hi, H)
        slab[:, :, :, slo - lo:shi - lo, :] = x[b, :, :, :, slo:shi, :]
        plane = np.zeros((C_IN, T, D, XH, XROW), np.float32)
        plane[:, :, :, :, 1:33] = slab
        flat = plane.reshape(C_IN, T, D * XDP)
        X = np.zeros((C_IN, T, XP), np.float16)
        X[:, :, 1:1 + D * XDP] = flat
        Xs.append(X)
        MTs.append(np.full((128, 1), 0.0 if j == 0 else 1.0, np.float32))
        MBs.append(np.full((128, 1), 0.0 if j == NJ - 1 else 1.0, np.float32))

    w1 = np.asarray(w1, np.float32)
    w2 = np.asarray(w2, np.float32)
    W1P = np.zeros((128, 27, 128), np.float16)   # tileA pairs: (kv,kw=0)+(kv,kw=1)
    W1PB = np.zeros((128, 9, 128), np.float16)   # tileB pair: (0,2)+(1,2)
    W1S = np.zeros((128, 9, 128), np.float16)    # tileB-top single: (2,2)
    for kt in range(3):
        for ku in range(3):
            g9 = kt * 3 + ku
            W1PB[:64, g9, :] = w1[:, :, kt, ku, 0, 2].T
            W1PB[64:, g9, :] = w1[:, :, kt, ku, 1, 2].T
            W1S[:64, g9, :] = w1[:, :, kt, ku, 2, 2].T
            for kv in range(3):
                g = _g27(kt, ku, kv)
                W1P[:64, g, :] = w1[:, :, kt, ku, kv, 0].T
                W1P[64:, g, :] = w1[:, :, kt, ku, kv, 1].T
    W2 = np.zeros((128, 81, 64), np.float16)
    for kt in range(3):
        for ku in range(3):
            for kv in range(3):
                for kw in range(3):
                    gi = _g81(kt, ku, kv, kw)
                    W2[:, gi, :] = w2[:, :, kt, ku, kv, kw].T
    return dict(X=Xs, MT=MTs, MB=MBs,
                W1P=W1P.reshape(128, 27 * 128), W1PB=W1PB.reshape(128, 9 * 128),
                W1S=W1S.reshape(128, 9 * 128),
                W2=W2.reshape(128, 81 * 64),
                B1=np.asarray(b1, np.float32).reshape(128, 1),
                B2=np.asarray(b2, np.float32).reshape(64, 1))


def _build_module():
    import concourse.bass as bass
    import concourse.tile as tile
    from concourse import bacc, mybir

    fp16 = mybir.dt.float16
    fp32 = mybir.dt.float32

    nc = bacc.Bacc("TRN2", target_bir_lowering=False, debug=False, num_devices=1)
    x_d = nc.dram_tensor("x", [64, T, XP], fp16, kind="ExternalInput")
    w1p_d = nc.dram_tensor("w1p", [128, 27 * 128], fp16, kind="ExternalInput")
    w1pb_d = nc.dram_tensor("w1pb", [128, 9 * 128], fp16, kind="ExternalInput")
    w1s_d = nc.dram_tensor("w1s", [128, 9 * 128], fp16, kind="ExternalInput")
    w2_d = nc.dram_tensor("w2", [128, 81 * 64], fp16, kind="ExternalInput")
    b1_d = nc.dram_tensor("b1", [128, 1], fp32, kind="ExternalInput")
    b2_d = nc.dram_tensor("b2", [64, 1], fp32, kind="ExternalInput")
    mt_d = nc.dram_tensor("mt", [128, 1], fp32, kind="ExternalInput")
    mb_d = nc.dram_tensor("mb", [128, 1], fp32, kind="ExternalInput")
    y_d = nc.dram_tensor("y", [64, T, D * SH * W], fp32, kind="ExternalOutput")

    with tile.TileContext(nc) as tc:
        with (
            tc.tile_pool(name="xw", bufs=1) as xw,
            tc.tile_pool(name="hp", bufs=1) as hpool,
            tc.tile_pool(name="st", bufs=4) as stp,
            tc.tile_pool(name="p1", bufs=4, space="PSUM") as p1,
            tc.tile_pool(name="p2", bufs=4, space="PSUM") as p2,
        ):
            w1p = xw.tile([128, 27, 128], fp16)
            nc.sync.dma_start(w1p[:, :, :], w1p_d.ap())
            b1 = xw.tile([128, 1], fp32)
            nc.sync.dma_start(b1[:, :], b1_d.ap())

            # tileA = (x, x+1), tileB = (x+2, x+36): shifted copies so each
            # K=128 matmul contracts two taps; quarter-chunked, first chunks
            # DMAd first so conv1 can start early
            xa = xw.tile([128, T, XP], fp16)
            xb = xw.tile([128, T, XP], fp16)
            qs = [0, XP // 4, XP // 2, 3 * XP // 4, XP]

            def xchunk(t, ci):
                lo, hi = qs[ci], qs[ci + 1]
                for tdst, p0, s in ((xa, 0, 0), (xa, 64, 1),
                                    (xb, 0, 2), (xb, 64, 36)):
                    he = min(hi, XP - s)
                    nc.sync.dma_start(tdst[p0:p0 + 64, t, lo:he],
                                      x_d.ap()[:, t, lo + s:he + s])

            # conv1 t=0 needs planes 0,1 and the tileB weights first
            xchunk(0, 0)
            xchunk(1, 0)
            w1pb = xw.tile([128, 9, 128], fp16)
            nc.sync.dma_start(w1pb[:, :, :], w1pb_d.ap())
            w1s = xw.tile([128, 9, 128], fp16)
            nc.sync.dma_start(w1s[:, :, :], w1s_d.ap())
            for t, ci in ((2, 0), (3, 0), (0, 1), (1, 1), (0, 2), (1, 2),
                          (0, 3), (1, 3), (2, 1), (3, 1), (2, 2), (3, 2),
                          (2, 3), (3, 3)):
                xchunk(t, ci)

            w2 = xw.tile([128, 81, 64], fp16)
            nc.sync.dma_start(w2[:, :, :], w2_d.ap())
            b2 = xw.tile([64, 1], fp32)
            nc.sync.dma_start(b2[:, :], b2_d.ap())
            mt = xw.tile([128, 1], fp32)
            nc.sync.dma_start(mt[:, :], mt_d.ap())
            mb = xw.tile([128, 1], fp32)
            nc.sync.dma_start(mb[:, :], mb_d.ap())

            ht = hpool.tile([128, T, HD, HHH, HW_], fp16)
            for t in range(T):
                nc.vector.memset(ht[:, t, :, :, :], 0.0)

            # ---- conv1 ----
            # per valid (kt, ku) block: 4 K=128 pairs + 1 K=64 single:
            #   tileA pairs at q=Bq+kv*34 cover (kv,kw=0)+(kv,kw=1)
            #   tileB pair  at q=Bq       covers (0,2)+(1,2)
            #   tileB-top single at q=Bq+68 covers (2,2)
            for t in range(T):
                for d in range(D):
                    blocks = [(kt, ku) for kt in _t_taps(t)
                              for ku in range(3) if 0 <= d + ku - 1 < D]
                    ps = p1.tile([128, HHH, XROW], fp32)
                    # all K=128 matmuls first, then all K=64 singles, so the
                    # PE sees only one tile_size transition per run (tile
                    # switches stall the LDWEIGHTS pipeline)
                    i = 0
                    for kt, ku in blocks:
                        tp = t + kt - 1
                        bq = (d + ku - 1) * XDP
                        for kv in range(3):
                            nc.tensor.matmul(
                                ps[:, :, :], w1p[:, _g27(kt, ku, kv), :],
                                xa[:, tp, bq + kv * XROW:bq + kv * XROW + N1],
                                start=(i == 0), stop=False)
                            i += 1
                        nc.tensor.matmul(
                            ps[:, :, :], w1pb[:, kt * 3 + ku, :],
                            xb[:, tp, bq:bq + N1],
                            start=False, stop=False)
                        i += 1
                    for i, (kt, ku) in enumerate(blocks):
                        tp = t + kt - 1
                        bq = (d + ku - 1) * XDP
                        nc.tensor.matmul(
                            ps[:, :, :], w1s[0:64, kt * 3 + ku, :],
                            xb[0:64, tp, bq + 68:bq + 68 + N1],
                            start=False, stop=(i == len(blocks) - 1))
                    nc.scalar.activation(
                        ht[:, t, d + 1, :, 1:33], ps[:, :, 1:33],
                        mybir.ActivationFunctionType.Relu, bias=b1[:, 0:1])
                # zero out-of-image h halo rows (mask is 0 only on edge cores)
                nc.vector.tensor_scalar_mul(
                    ht[:, t, :, 0, 1:33], ht[:, t, :, 0, 1:33], mt[:, 0:1])
                nc.vector.tensor_scalar_mul(
                    ht[:, t, :, HHH - 1, 1:33], ht[:, t, :, HHH - 1, 1:33],
                    mb[:, 0:1])

            # ---- conv2 ----
            # runs: edge d=0 and d=15 alone (N=256, zero-pad taps skipped),
            # interior d as 7 pairs (N=512). Taps alternate between PE column
            # groups (psum partitions 0:64 / 64:128) so adjacent matmuls run
            # concurrently; halves summed via Scalar+DVE into the stage tile.
            runs = [(0, 1)] + [(d0, 2) for d0 in range(1, 15, 2)] + [(15, 1)]
            for t in range(T):
                for d0, nd in runs:
                    taps = [(kt, ku, kv, kw) for kt in _t_taps(t)
                            for ku in range(3) if 0 < d0 + ku < 17 or nd == 2
                            for kv in range(3) for kw in range(3)]
                    nn = nd * SH * W
                    lo = taps[0::2]
                    hi = taps[1::2]
                    ps = p2.tile([128, N2], fp32)
                    for i in range(len(lo)):
                        for half, base, tp_pos in ((lo, 0, (0, 0)),
                                                   (hi, 64, (0, 64))):
                            if i >= len(half):
                                continue
                            kt, ku, kv, kw = half[i]
                            gi = _g81(kt, ku, kv, kw)
                            rhs = ht[:, t + kt - 1, d0 + ku:d0 + ku + nd,
                                     kv:kv + SH, kw:kw + W]
                            nc.tensor.matmul(
                                ps[base:base + 64, 0:nn], w2[:, gi, :], rhs,
                                start=(i == 0), stop=(i == len(half) - 1),
                                tile_position=tp_pos)
                    st = stp.tile([64, N2], fp32)
                    nc.scalar.activation(
                        st[:, 0:nn], ps[64:128, 0:nn],
                        mybir.ActivationFunctionType.Identity, bias=b2[:, 0:1])
                    nc.vector.tensor_add(st[:, 0:nn], st[:, 0:nn],
                                         ps[0:64, 0:nn])
                    nc.sync.dma_start(
                        y_d.ap()[:, t, d0 * SH * W:d0 * SH * W + nn],
                        st[:, 0:nn])
    nc.compile()
    return nc


def kernel(x, w1, b1, w2, b2):
    from concourse.bass_utils import run_bass_kernel_spmd

    hostd = _make_host_arrays(x, w1, b1, w2, b2)
    if "nc" not in _cache:
        _cache["nc"] = _build_module()
    nc = _cache["nc"]

    in_maps = []
    for core in range(NCORES):
        in_maps.append({
            "x": hostd["X"][core], "mt": hostd["MT"][core],
            "mb": hostd["MB"][core],
            "w1p": hostd["W1P"], "w1pb": hostd["W1PB"],
            "w1s": hostd["W1S"], "w2": hostd["W2"],
            "b1": hostd["B1"], "b2": hostd["B2"],
        })
    res = run_bass_kernel_spmd(nc, in_maps, core_ids=list(range(NCORES)))

    y = np.zeros((B, C_OUT, T, D, H, W), np.float32)
    for core in range(NCORES):
        b, j = divmod(core, NJ)
        yc = res.results[core]["y"].reshape(C_OUT, T, D, SH, W)
        y[b, :, :, :, SH * j:SH * (j + 1), :] = yc
    return y



# revision 5
# speedup vs baseline: 1.3492x; 1.3492x over previous
"""4D Conv-MLP (conv3^4 -> ReLU -> conv3^4) on 8 Trainium2 NeuronCores.

Sharding: core = b*4 + j (batch b in {0,1}, H-slab j in {0..3}, 8 output rows
each). Conv1 is recomputed on a 1-row h halo (10 h rows from 12 x rows), so no
cross-core communication. One SPMD program; boundary behavior is data-driven
(host-zeroed x halos + h halo-row masks).

Winograd F(2,3) along W on both convs (1.5x fewer multiplies): the 3 kw taps
become 4 pointwise components m0..m3 evaluated at 16 stride-2 w-tiles;
y[2k] = m0+m1+m2, y[2k+1] = m1-m2-m3. T/D/H taps (kt, ku, kv) stay direct.

  - x~ (input transform) is built on host: 4 component planes [T, D, 12, 16],
    components packed in pairs on 128 partitions (xA = c0|c1, xB = c2|c3).
  - conv1: K=64 matmuls, one PSUM accumulator chain per component; adjacent
    components alternate PE row-halves via tile_position (0,0)/(64,0) so two
    matmuls co-stream (2 cols/cycle aggregate). d processed in runs of <=3
    (N <= 480 <= one PSUM bank).
  - Inverse transform + ReLU + bias on DVE/Scalar writes h_e/h_o staging;
    h~ (conv2's forward transform, 4 components) is built from them with
    shifted adds and stored for all (t, d): [128, T, 4, D, 10, 16] fp16.
  - conv2: K=128 M=64 matmuls; components alternate PE column-halves via
    tile_position (0,0)/(0,64) (co-stream, as in the direct kernel). d runs
    of <=4 (N <= 512). Inverse + bias on DVE/Scalar, stride-2 DMA to y.
All matmul operands fp16, PSUM accumulation fp32. The PE is stream-bound
(LDWEIGHTS fully pipelines), so wall ~ total stream columns / 2 streams.
"""

import numpy as np

B, C_IN, C_HID, C_OUT = 2, 64, 128, 64
T, D, H, W = 4, 16, 32, 32
NCORES, NJ = 8, 4
SH = H // NJ          # 8 out rows per slab
XH = SH + 4           # 12 x rows per slab
HR = SH + 2           # 10 h rows per slab (1-row halo each side)
KW = W // 2           # 16 winograd w-tiles
XPL = D * XH * KW     # x~ plane per t = 3072
HPL = D * HR * KW     # h~ plane per (t, comp) = 2560

# d-runs: (dlo, nd); valid ku for a run = [max(0,1-dlo), min(2, 17-dlo-nd)]
RUNS1 = [(0, 1), (1, 3), (4, 3), (7, 3), (10, 3), (13, 2), (15, 1)]
RUNS2 = [(0, 1), (1, 4), (5, 4), (9, 4), (13, 2), (15, 1)]

_cache = {}


def _t_taps(t):
    return [kt for kt in range(3) if 0 <= t + kt - 1 < T]


def _ku_valid(dlo, nd):
    return [ku for ku in range(3) if dlo + ku - 1 >= 0 and dlo + nd + ku - 2 <= 15]


def _g27(kt, ku, kv):
    return (kt * 3 + ku) * 3 + kv


def _wino_w(g):
    """F(2,3) weight transform along the last axis (len 3) -> 4 components."""
    c0 = g[..., 0]
    c1 = 0.5 * (g[..., 0] + g[..., 1] + g[..., 2])
    c2 = 0.5 * (g[..., 0] - g[..., 1] + g[..., 2])
    c3 = g[..., 2]
    return c0, c1, c2, c3


def _make_host_arrays(x, w1, b1, w2, b2):
    x = np.asarray(x, np.float32)
    XAs, XBs, MTs, MBs = [], [], [], []
    for core in range(NCORES):
        b, j = divmod(core, NJ)
        h0 = SH * j
        slab = np.zeros((C_IN, T, D, XH, W + 3), np.float32)
        lo, hi = h0 - 2, h0 + 10
        slo, shi = max(lo, 0), min(hi, H)
        slab[:, :, :, slo - lo:shi - lo, 1:33] = x[b, :, :, :, slo:shi, :]
        # winograd input transform along W: tile k reads slab cols 2k..2k+3
        d0 = slab[..., 0:32:2]
        d1 = slab[..., 1:33:2]
        d2 = slab[..., 2:34:2]
        d3 = slab[..., 3:35:2]
        xt0 = d0 - d2
        xt1 = d1 + d2
        xt2 = d2 - d1
        xt3 = d1 - d3
        XAs.append(np.concatenate([xt0, xt1], 0).astype(np.float16)
                   .reshape(128, T, XPL))
        XBs.append(np.concatenate([xt2, xt3], 0).astype(np.float16)
                   .reshape(128, T, XPL))
        MTs.append(np.full((128, 1), 0.0 if j == 0 else 1.0, np.float32))
        MBs.append(np.full((128, 1), 0.0 if j == NJ - 1 else 1.0, np.float32))

    w1 = np.asarray(w1, np.float32)   # [128, 64, 3,3,3,3]
    w2 = np.asarray(w2, np.float32)   # [64, 128, 3,3,3,3]
    W1A = np.zeros((128, 27, 128), np.float16)
    W1B = np.zeros((128, 27, 128), np.float16)
    W2T = np.zeros((128, 4, 27, 64), np.float16)
    for kt in range(3):
        for ku in range(3):
            for kv in range(3):
                g = _g27(kt, ku, kv)
                c1s = _wino_w(w1[:, :, kt, ku, kv, :])   # each [128out, 64in]
                W1A[0:64, g, :] = c1s[0].T
                W1A[64:128, g, :] = c1s[1].T
                W1B[0:64, g, :] = c1s[2].T
                W1B[64:128, g, :] = c1s[3].T
                c2s = _wino_w(w2[:, :, kt, ku, kv, :])   # each [64out, 128in]
                for c in range(4):
                    W2T[:, c, g, :] = c2s[c].T
    return dict(XA=XAs, XB=XBs, MT=MTs, MB=MBs,
                W1A=W1A.reshape(128, 27 * 128), W1B=W1B.reshape(128, 27 * 128),
                W2=W2T.reshape(128, 4 * 27 * 64),
                B1=np.asarray(b1, np.float32).reshape(128, 1),
                B2=np.asarray(b2, np.float32).reshape(64, 1))


def _build_module():
    import concourse.tile as tile
    from concourse import bacc, mybir

    fp16 = mybir.dt.float16
    fp32 = mybir.dt.float32
    RELU = mybir.ActivationFunctionType.Relu
    IDENT = mybir.ActivationFunctionType.Identity

    nc = bacc.Bacc("TRN2", target_bir_lowering=False, debug=False, num_devices=1)
    xa_d = nc.dram_tensor("xa", [128, T, XPL], fp16, kind="ExternalInput")
    xb_d = nc.dram_tensor("xb", [128, T, XPL], fp16, kind="ExternalInput")
    w1a_d = nc.dram_tensor("w1a", [128, 27 * 128], fp16, kind="ExternalInput")
    w1b_d = nc.dram_tensor("w1b", [128, 27 * 128], fp16, kind="ExternalInput")
    w2_d = nc.dram_tensor("w2", [128, 4 * 27 * 64], fp16, kind="ExternalInput")
    b1_d = nc.dram_tensor("b1", [128, 1], fp32, kind="ExternalInput")
    b2_d = nc.dram_tensor("b2", [64, 1], fp32, kind="ExternalInput")
    mt_d = nc.dram_tensor("mt", [128, 1], fp32, kind="ExternalInput")
    mb_d = nc.dram_tensor("mb", [128, 1], fp32, kind="ExternalInput")
    # even / odd w-planes stored separately; host interleaves
    y_d = nc.dram_tensor("y", [64, T, 2, D * SH * KW], fp32,
                         kind="ExternalOutput")

    with tile.TileContext(nc) as tc:
        with (
            tc.tile_pool(name="xw", bufs=1) as xw,
            tc.tile_pool(name="st", bufs=2) as stp,
            tc.tile_pool(name="pp", bufs=2, space="PSUM") as pp,
        ):
            w1a = xw.tile([128, 27, 128], fp16)
            nc.sync.dma_start(w1a[:, :, :], w1a_d.ap())
            w1b = xw.tile([128, 27, 128], fp16)
            nc.sync.dma_start(w1b[:, :, :], w1b_d.ap())
            b1 = xw.tile([128, 1], fp32)
            nc.sync.dma_start(b1[:, :], b1_d.ap())
            mt = xw.tile([128, 1], fp32)
            nc.sync.dma_start(mt[:, :], mt_d.ap())
            mb = xw.tile([128, 1], fp32)
            nc.sync.dma_start(mb[:, :], mb_d.ap())

            xA = xw.tile([128, T, D, XH, KW], fp16)
            xB = xw.tile([128, T, D, XH, KW], fp16)

            def xchunk(t):
                nc.sync.dma_start(xA[:, t, :, :, :], xa_d.ap()[:, t, :])
                nc.sync.dma_start(xB[:, t, :, :, :], xb_d.ap()[:, t, :])

            xchunk(0)
            xchunk(1)
            xchunk(2)
            xchunk(3)

            w2t = xw.tile([128, 4, 27, 64], fp16)
            nc.sync.dma_start(w2t[:, :, :, :], w2_d.ap())
            b2 = xw.tile([64, 1], fp32)
            nc.sync.dma_start(b2[:, :], b2_d.ap())

            hT = xw.tile([128, T, 4, D, HR, KW], fp16)

            # prime the h_e/h_o staging buffers so their pad cols stay zero
            for _ in range(2):
                he = stp.tile([128, 3, HR, 18], fp16)
                nc.vector.memset(he[:, :, :, :], 0.0)
                ho = stp.tile([128, 3, HR, 18], fp16)
                nc.vector.memset(ho[:, :, :, :], 0.0)

            # ---- conv1 (winograd-W components, K=64 row-split co-stream) ----
            for t in range(T):
                kts = _t_taps(t)
                for dlo, nd in RUNS1:
                    kus = _ku_valid(dlo, nd)
                    n = nd * HR * KW
                    ps0 = pp.tile([128, 512], fp32)
                    ps1 = pp.tile([128, 512], fp32)
                    ps2 = pp.tile([128, 512], fp32)
                    ps3 = pp.tile([128, 512], fp32)
                    pss = (ps0, ps1, ps2, ps3)
                    taps = [(kt, ku, kv) for kt in kts for ku in kus
                            for kv in range(3)]
                    for i, (kt, ku, kv) in enumerate(taps):
                        tp = t + kt - 1
                        dp = dlo + ku - 1
                        g = _g27(kt, ku, kv)
                        st_f = (i == 0)
                        sp_f = (i == len(taps) - 1)
                        for c, (xt, wt) in enumerate(((xA, w1a), (xA, w1a),
                                                      (xB, w1b), (xB, w1b))):
                            p0 = 64 * (c % 2)
                            nc.tensor.matmul(
                                pss[c][:, 0:n], wt[p0:p0 + 64, g, :],
                                xt[p0:p0 + 64, tp, dp:dp + nd, kv:kv + HR, :],
                                start=st_f, stop=sp_f, tile_position=(p0, 0))
                    # inverse: h_even = ReLU(m0+m1+m2+b1), h_odd = ReLU(m1-m2-m3+b1)
                    cst = stp.tile([128, 512], fp32)
                    nc.scalar.activation(cst[:, 0:n], ps1[:, 0:n], IDENT)
                    ust = stp.tile([128, 512], fp32)
                    nc.vector.tensor_add(ust[:, 0:n], cst[:, 0:n], ps2[:, 0:n])
                    vst = stp.tile([128, 512], fp32)
                    nc.vector.tensor_sub(vst[:, 0:n], cst[:, 0:n], ps2[:, 0:n])
                    nc.vector.tensor_add(ust[:, 0:n], ust[:, 0:n], ps0[:, 0:n])
                    nc.vector.tensor_sub(vst[:, 0:n], vst[:, 0:n], ps3[:, 0:n])
                    he = stp.tile([128, 3, HR, 18], fp16)
                    nc.scalar.activation(he[:, 0:nd, :, 1:17],
                                         ust[:, 0:n], RELU, bias=b1[:, 0:1])
                    ho = stp.tile([128, 3, HR, 18], fp16)
                    nc.scalar.activation(ho[:, 0:nd, :, 1:17],
                                         vst[:, 0:n], RELU, bias=b1[:, 0:1])
                    # zero out-of-image h halo rows (edge cores only)
                    nc.vector.tensor_scalar_mul(
                        he[:, 0:nd, 0, 1:17], he[:, 0:nd, 0, 1:17], mt[:, 0:1])
                    nc.vector.tensor_scalar_mul(
                        ho[:, 0:nd, 0, 1:17], ho[:, 0:nd, 0, 1:17], mt[:, 0:1])
                    nc.vector.tensor_scalar_mul(
                        he[:, 0:nd, HR - 1, 1:17], he[:, 0:nd, HR - 1, 1:17],
                        mb[:, 0:1])
                    nc.vector.tensor_scalar_mul(
                        ho[:, 0:nd, HR - 1, 1:17], ho[:, 0:nd, HR - 1, 1:17],
                        mb[:, 0:1])
                    # h~ components: h~0=ho[k-1]-ho[k], h~1=he+ho, h~2=ho-he,
                    #                h~3=he[k]-he[k+1]
                    nc.vector.tensor_sub(hT[:, t, 0, dlo:dlo + nd, :, :],
                                         ho[:, 0:nd, :, 0:16],
                                         ho[:, 0:nd, :, 1:17])
                    nc.vector.tensor_add(hT[:, t, 1, dlo:dlo + nd, :, :],
                                         he[:, 0:nd, :, 1:17],
                                         ho[:, 0:nd, :, 1:17])
                    nc.vector.tensor_sub(hT[:, t, 2, dlo:dlo + nd, :, :],
                                         ho[:, 0:nd, :, 1:17],
                                         he[:, 0:nd, :, 1:17])
                    nc.vector.tensor_sub(hT[:, t, 3, dlo:dlo + nd, :, :],
                                         he[:, 0:nd, :, 1:17],
                                         he[:, 0:nd, :, 2:18])

            # ---- conv2 (winograd-W components, K=128 col-split co-stream) ----
            for t in range(T):
                kts = _t_taps(t)
                for dlo, nd in RUNS2:
                    kus = _ku_valid(dlo, nd)
                    n = nd * SH * KW
                    ps0 = pp.tile([128, 512], fp32)
                    ps1 = pp.tile([128, 512], fp32)
                    taps = [(kt, ku, kv) for kt in kts for ku in kus
                            for kv in range(3)]
                    for i, (kt, ku, kv) in enumerate(taps):
                        tp = t + kt - 1
                        dp = dlo + ku - 1
                        g = _g27(kt, ku, kv)
                        st_f = (i == 0)
                        sp_f = (i == len(taps) - 1)
                        for c in range(4):
                            ps = ps0 if c < 2 else ps1
                            base = 64 * (c % 2)
                            nc.tensor.matmul(
                                ps[base:base + 64, 0:n], w2t[:, c, g, :],
                                hT[:, tp, c, dp:dp + nd, kv:kv + SH, :],
                                start=st_f, stop=sp_f, tile_position=(0, base))
                    # inverse: y_even = m0+m1+m2+b2, y_odd = m1-m2-m3+b2
                    cst = stp.tile([64, 512], fp32)
                    nc.scalar.activation(cst[:, 0:n], ps0[64:128, 0:n], IDENT)
                    ust = stp.tile([128, 512], fp32)
                    nc.vector.tensor_add(ust[0:64, 0:n], cst[:, 0:n],
                                         ps1[0:64, 0:n])
                    vst = stp.tile([128, 512], fp32)
                    nc.vector.tensor_sub(vst[0:64, 0:n], cst[:, 0:n],
                                         ps1[0:64, 0:n])
                    nc.vector.tensor_add(ust[0:64, 0:n], ust[0:64, 0:n],
                                         ps0[0:64, 0:n])
                    nc.vector.tensor_sub(vst[0:64, 0:n], vst[0:64, 0:n],
                                         ps1[64:128, 0:n])
                    ye2 = stp.tile([64, 512], fp32)
                    nc.scalar.activation(ye2[:, 0:n], ust[0:64, 0:n], IDENT,
                                         bias=b2[:, 0:1])
                    yo2 = stp.tile([64, 512], fp32)
                    nc.scalar.activation(yo2[:, 0:n], vst[0:64, 0:n], IDENT,
                                         bias=b2[:, 0:1])
                    base = dlo * SH * KW
                    nc.sync.dma_start(y_d.ap()[:, t, 0, base:base + n],
                                      ye2[:, 0:n])
                    nc.sync.dma_start(y_d.ap()[:, t, 1, base:base + n],
                                      yo2[:, 0:n])
    nc.compile()
    return nc


def kernel(x, w1, b1, w2, b2):
    from concourse.bass_utils import run_bass_kernel_spmd

    hostd = _make_host_arrays(x, w1, b1, w2, b2)
    if "nc" not in _cache:
        _cache["nc"] = _build_module()
    nc = _cache["nc"]

    in_maps = []
    for core in range(NCORES):
        in_maps.append({
            "xa": hostd["XA"][core], "xb": hostd["XB"][core],
            "mt": hostd["MT"][core], "mb": hostd["MB"][core],
            "w1a": hostd["W1A"], "w1b": hostd["W1B"], "w2": hostd["W2"],
            "b1": hostd["B1"], "b2": hostd["B2"],
        })
    res = run_bass_kernel_spmd(nc, in_maps, core_ids=list(range(NCORES)))

    y = np.zeros((B, C_OUT, T, D, H, W), np.float32)
    for core in range(NCORES):
        b, j = divmod(core, NJ)
        yc = res.results[core]["y"].reshape(C_OUT, T, 2, D, SH, KW)
        ys = y[b, :, :, :, SH * j:SH * (j + 1), :]
        ys[..., 0::2] = yc[:, :, 0]
        ys[..., 1::2] = yc[:, :, 1]
    return y
